# revision 2
# baseline (speedup 1.0000x reference)
"""Trainium2 Bass kernel for causal self-attention with T5 relative position bias.

Problem (hardcoded): B=4, T=2048, C=1024, H=16, D=64, NUM_BUCKETS=32, MAX_DISTANCE=128.
Sharding over 8 cores: core c -> (batch b=c//2, head-group hg=c%2 of 8 heads).
Each core computes qkv projection for its heads, causal attention, and a partial
output projection (its heads' rows of W_proj); host sums the two partials per batch.

On-chip layout notes:
  - x, q, k are kept transposed ([C, T]-style, channel on partitions) so every
    matmul contracts over the partition dim with no on-chip transposes.
  - Attention logits are computed transposed: S_T[tk, tq] = k_h^T q_h (K=64).
  - Softmax skips max-subtraction (logits ~ N(0,1); exp <= e^7 fits fp16 easily).
  - The T5 bias + causal mask are folded into one fp16 Toeplitz table per head:
    expAm[p, x] = exp(bias[d]) * (d >= 0), d = x - p - 384.  P = exp(S/8) * expAm.
    Tables are expanded on the host (strided DMA reads decompose into
    per-element descriptors and run ~100x slower than contiguous loads).
  - Row sums come free from a ones-column appended to V (AV matmul M=65).
    Normalization: batched 4-lane reciprocal per head, broadcast across
    partitions by bouncing the row through DRAM and reading it back with a
    stride-0 partition step (legal on the DRAM side only).
"""

import sys

sys.path.insert(0, "/opt/trn_rl_repo")

import math

import numpy as np

import concourse.bacc as bacc
import concourse.bass as bass
import concourse.mybir as mybir
import concourse.tile as tile
from concourse import bass_utils


def _ensure_axon_hooks():
    """bass_utils imports antenv.axon_hooks when BASS_TRACE is set under axon;
    this image's antenv lacks that submodule. Provide an inert one so a stray
    trace env var degrades to a warning instead of crashing the run."""
    try:
        import antenv.axon_hooks  # noqa: F401
    except Exception:
        try:
            import types

            import antenv

            hooks = types.ModuleType("antenv.axon_hooks")
            hooks._hook = None
            hooks.set_axon_ntff_profile_hook = lambda h: setattr(hooks, "_hook", h)
            hooks.get_axon_ntff_profile_hook = lambda: hooks._hook
            sys.modules["antenv.axon_hooks"] = hooks
            antenv.axon_hooks = hooks
        except Exception:
            pass


_ensure_axon_hooks()

B, T, C = 4, 2048, 1024
H, D = 16, 64
NUM_BUCKETS, MAX_DISTANCE = 32, 128
HL = 8  # local heads per core
CL = HL * D  # 512 local channels
NCORES = 8

FP16 = mybir.dt.float16
FP32 = mybir.dt.float32

# expAm table geometry: slice start s = (tq0 - tk0) + 384 in [0, 1920], width 512
EA_W = 2432  # 1920 + 512
EA_VEC = EA_W + 127  # 2559: w[j] = exp(bias[j - 511]) masked, j-index = d + 511


def _build_program(sim_safe=False):
    """sim_safe=True keeps the AV matmuls full-width so CoreSim's PSUM
    accumulation-group tracker stays happy (narrowed AV is correct on HW:
    has_written is per element, and every pav element is written by the j=0
    full-width matmul before any read)."""
    nc = bacc.Bacc(None, target_bir_lowering=False)

    xT = nc.dram_tensor("xT", [C, T], FP16, kind="ExternalInput")
    wq = nc.dram_tensor("wq", [C, CL], FP16, kind="ExternalInput")
    wk = nc.dram_tensor("wk", [C, CL], FP16, kind="ExternalInput")
    wv = nc.dram_tensor("wv", [C, CL], FP16, kind="ExternalInput")
    wp = nc.dram_tensor("wp", [CL, C], FP16, kind="ExternalInput")
    bqk = nc.dram_tensor("bqk", [2, CL], FP32, kind="ExternalInput")
    bvr = nc.dram_tensor("bvr", [128, CL], FP32, kind="ExternalInput")
    wexp = nc.dram_tensor("wexp", [HL, 128, EA_W], FP16, kind="ExternalInput")
    yp = nc.dram_tensor("yp", [C, T], FP32, kind="ExternalOutput")
    # DRAM scratch rows for the reciprocal-row broadcast (one per head x chunk)
    rscratch = nc.dram_tensor("rscratch", [HL * 4, 512], FP16)

    NT = T // 512  # 4 tq/t chunks of 512
    NK = T // 128  # 16 tk/t chunks of 128
    KC = C // 128  # 8 contraction chunks for qkv
    MC = CL // 128  # 4 m-chunks of local channels

    with tile.TileContext(nc) as tc:
        with (
            tc.tile_pool(name="w", bufs=1) as wpool,
            tc.tile_pool(name="big", bufs=1) as bigpool,
            tc.tile_pool(name="ea", bufs=3) as eapool,
            tc.tile_pool(name="tr", bufs=4) as tr,
            tc.tile_pool(name="sm", bufs=2) as smpool,
            tc.tile_pool(name="ev", bufs=6) as evpool,
            tc.tile_pool(name="ps", bufs=2, space="PSUM") as ps,
            tc.tile_pool(name="psav", bufs=4, space="PSUM") as psav,
        ):
            # ---- weights / constants ----
            # DMA order: first q-matmul inputs (x chunk 0, wq, bq) land first
            wq_sb = wpool.tile([128, KC, CL], FP16)
            wk_sb = wpool.tile([128, KC, CL], FP16)
            wv_sb = wpool.tile([128, KC, CL], FP16)
            wp_sb = wpool.tile([128, MC, C], FP16)
            bq_sb = wpool.tile([128, MC], FP32)
            bk_sb = wpool.tile([128, MC], FP32)
            bv_sb = wpool.tile([128, CL], FP32)
            xt_sb = bigpool.tile([128, KC, T], FP16)
            xr = xT.rearrange("(kc p) (tc t) -> p kc tc t", p=128, t=512)
            bqk_r = bqk.rearrange("b (m p) -> b p m", p=128)

            # kc-granular first loads: the first matmul only waits for its
            # own 128KB slices instead of two 1MB transfers
            wq_r = wq.rearrange("(kc p) m -> p kc m", p=128)
            nc.sync.dma_start(out=bq_sb, in_=bqk_r[0])
            for kc in range(KC):
                nc.sync.dma_start(
                    out=xt_sb[:, kc, 0:512], in_=xr[:, kc, 0]
                )
                nc.sync.dma_start(out=wq_sb[:, kc], in_=wq_r[:, kc])
            nc.sync.dma_start(out=wk_sb, in_=wk.rearrange("(kc p) m -> p kc m", p=128))
            nc.sync.dma_start(out=bk_sb, in_=bqk_r[1])
            nc.sync.dma_start(out=wv_sb, in_=wv.rearrange("(kc p) m -> p kc m", p=128))
            nc.sync.dma_start(out=bv_sb, in_=bvr[:])
            for tch in range(1, NT):
                nc.sync.dma_start(
                    out=xt_sb[:, :, tch * 512 : (tch + 1) * 512], in_=xr[:, :, tch]
                )
            nc.sync.dma_start(out=wp_sb, in_=wp.rearrange("(kc p) m -> p kc m", p=128))

            # ---- persistent activations ----
            qT_sb = bigpool.tile([128, MC, T], FP16)  # c' = m*128 + p
            kT_sb = bigpool.tile([128, MC, T], FP16)
            v_sb = bigpool.tile([128, NK, HL * 65], FP16)  # slot l: [v(64), ones]
            y_sb = bigpool.tile([128, MC, T], FP16)  # y_cat_T, c_in = m*128 + p

            for l in range(HL):
                nc.vector.memset(v_sb[:, :, l * 65 + 64 : l * 65 + 65], 1.0)

            # ---- stage 1: qkv projections ----
            for tch in range(NT):
                tsl = slice(tch * 512, (tch + 1) * 512)
                for m in range(MC):
                    msl = slice(m * 128, (m + 1) * 128)
                    pq = psav.tile([128, 512], FP32, tag="pav")
                    for kc in range(KC):
                        nc.tensor.matmul(
                            pq[:],
                            wq_sb[:, kc, msl],
                            xt_sb[:, kc, tsl],
                            start=(kc == 0),
                            stop=(kc == KC - 1),
                        )
                    nc.vector.tensor_scalar_add(
                        out=qT_sb[:, m, tsl], in0=pq[:], scalar1=bq_sb[:, m : m + 1],
                    )
                    pk = psav.tile([128, 512], FP32, tag="pav")
                    for kc in range(KC):
                        nc.tensor.matmul(
                            pk[:],
                            wk_sb[:, kc, msl],
                            xt_sb[:, kc, tsl],
                            start=(kc == 0),
                            stop=(kc == KC - 1),
                        )
                    nc.vector.tensor_scalar_add(
                        out=kT_sb[:, m, tsl], in0=pk[:], scalar1=bk_sb[:, m : m + 1],
                    )
                # v: plain layout [t, c'] so AV's lhsT has tk on partitions
                for ts in range(4):
                    t16 = tch * 4 + ts
                    pv = psav.tile([128, 512], FP32, tag="pav")
                    for kc in range(KC):
                        nc.tensor.matmul(
                            pv[:],
                            xt_sb[:, kc, t16 * 128 : (t16 + 1) * 128],
                            wv_sb[:, kc, :],
                            start=(kc == 0),
                            stop=(kc == KC - 1),
                        )
                    # scatter into 65-wide slots (even/odd strided copies) + bias
                    for par in range(2):
                        src = bass.AP(
                            tensor=pv.tensor, offset=pv.offset + par * 64,
                            ap=[pv.ap[0], [128, 4], [1, 64]],
                        )
                        srcb = bass.AP(
                            tensor=bv_sb.tensor, offset=bv_sb.offset + par * 64,
                            ap=[bv_sb.ap[0], [128, 4], [1, 64]],
                        )
                        base = v_sb[:, t16]
                        dst = bass.AP(
                            tensor=base.tensor, offset=base.offset + par * 65,
                            ap=[base.ap[0], [130, 4], [1, 64]],
                        )
                        nc.vector.tensor_add(out=dst, in0=src, in1=srcb)

            # ---- stage 2: attention per local head ----
            for l in range(HL):
                pb = (l % 2) * 64
                mq = l // 2
                # host-expanded Toeplitz table (strided/reversed DMA reads decompose
                # into per-element descriptors and take ~300us; a plain contiguous
                # 600KB DMA takes ~2us)
                ea_sb = eapool.tile([128, EA_W], FP16, tag="ea")
                nc.sync.dma_start(out=ea_sb, in_=wexp[l])

                rsg32 = smpool.tile([4, 512], FP32, tag="rsg")
                yevs = [None] * NT
                corder = range(NT - 1, -1, -1) if (l == 0 and not sim_safe) else range(NT)
                for c in corder:
                    nj = 4 * c + 4
                    nfull = 4 * c + 1  # tiles with off == 0
                    pav = psav.tile([65, 512], FP32, tag="pav")

                    def av(j, pm_ap, avsl):
                        nc.tensor.matmul(
                            pav[:, avsl],
                            v_sb[:, j, l * 65 : l * 65 + 65],
                            pm_ap,
                            start=(j == 0),
                            stop=(j == nj - 1),
                        )

                    if not sim_safe:
                        # ALL tiles in pairs: two S matmuls into one 2-bank
                        # PSUM tile, one exp / mask-mult over [128, 1024]
                        # (ACTIVATE has a 352-cycle fixed overhead). The
                        # pair's ea slices start 128 apart, so an overlapped
                        # [[128, 2], [1, 512]] read covers both; ea's zeros
                        # blank each half's causally-masked columns.
                        for i in range(nj // 2):
                            ja, jb = 2 * i, 2 * i + 1
                            off_a = max(0, 128 * ja - 512 * c)
                            off_b = max(0, 128 * jb - 512 * c)
                            s_b = 512 * c - 128 * jb + 384
                            pS2 = ps.tile([128, 1024], FP32, tag="pS2")
                            nc.tensor.matmul(
                                pS2[:, off_b:512],
                                kT_sb[pb : pb + 64, mq, jb * 128 : (jb + 1) * 128],
                                qT_sb[pb : pb + 64, mq, c * 512 + off_b : (c + 1) * 512],
                                start=True, stop=True,
                            )
                            nc.tensor.matmul(
                                pS2[:, 512 + off_a : 1024],
                                kT_sb[pb : pb + 64, mq, ja * 128 : (ja + 1) * 128],
                                qT_sb[pb : pb + 64, mq, c * 512 + off_a : (c + 1) * 512],
                                start=True, stop=True,
                            )
                            p2 = tr.tile([128, 1024], FP16, tag="p")
                            nc.scalar.activation(
                                out=p2[:, off_b:1024], in_=pS2[:, off_b:1024],
                                func=mybir.ActivationFunctionType.Exp,
                                scale=1.0 / math.sqrt(D),
                            )
                            pm2 = tr.tile([128, 1024], FP16, tag="pm")
                            ea_pair = bass.AP(
                                tensor=ea_sb.tensor,
                                offset=ea_sb.offset + s_b,
                                ap=[ea_sb.ap[0], [128, 2], [1, 512]],
                            )
                            nc.vector.tensor_mul(
                                out=pm2.rearrange("p (a n) -> p a n", a=2),
                                in0=p2.rearrange("p (a n) -> p a n", a=2),
                                in1=ea_pair,
                            )
                            av(ja, pm2[:, 512 + off_a : 1024], slice(off_a, 512))
                            av(jb, pm2[:, off_b:512], slice(off_b, 512))
                    else:
                        # sim-only variant: unpaired, fully-initialized tiles
                        # (CoreSim race/accumulation trackers reject the stale
                        # never-read columns the HW path leaves behind)
                        for j in range(nj):
                            off = max(0, 128 * j - 512 * c)
                            csl = slice(off, 512)
                            s_off = 512 * c - 128 * j + 384 + off
                            pS = ps.tile([128, 1024], FP32, tag="pS2")
                            nc.tensor.matmul(
                                pS[:, csl],
                                kT_sb[pb : pb + 64, mq, j * 128 : (j + 1) * 128],
                                qT_sb[pb : pb + 64, mq, c * 512 + off : (c + 1) * 512],
                                start=True, stop=True,
                            )
                            p_sb = tr.tile([128, 1024], FP16, tag="p")
                            nc.scalar.activation(
                                out=p_sb[:, csl], in_=pS[:, csl],
                                func=mybir.ActivationFunctionType.Exp,
                                scale=1.0 / math.sqrt(D),
                            )
                            pm_sb = tr.tile([128, 1024], FP16, tag="pm")
                            if off:
                                nc.gpsimd.memset(pm_sb[:, 0:off], 0.0)
                            nc.vector.tensor_mul(
                                out=pm_sb[:, csl], in0=p_sb[:, csl],
                                in1=ea_sb[:, s_off : s_off + 512 - off],
                            )
                            av(j, pm_sb[:, 0:512], slice(0, 512))

                    # evacuate pav to SBUF with one ACT copy so the PSUM slot
                    # frees immediately (the serialized DVE reciprocals were
                    # stalling the next head's AV matmuls at ~3.3us each)
                    yev = evpool.tile([128, 512], FP32, tag="yev")
                    nc.vector.tensor_copy(yev[0:65, :], pav[0:65, :])
                    yevs[c] = yev
                    if l < HL - 1:
                        # gather the rowsum row into partition c of a [4, 512]
                        # tile so one 4-lane reciprocal serves the whole head
                        nc.sync.dma_start(out=rsg32[c : c + 1, :], in_=yev[64:65, :])
                    else:
                        # last head: per-chunk reciprocal straight off yev so
                        # chunk 0 normalizes while chunks 1-3 still compute
                        # and the projection can start early
                        rc32 = smpool.tile([128, 512], FP32, tag="rec32")
                        nc.vector.reciprocal(out=rc32[64:65, :], in_=yev[64:65, :])
                        rc16 = smpool.tile([128, 512], FP16, tag="rec16")
                        nc.vector.tensor_copy(rc16[64:65, :], rc32[64:65, :])
                        nc.sync.dma_start(out=rscratch[l * 4 + c], in_=rc16[64:65, :])

                # normalize: y[c] = yev[c][0:64] * broadcast(1 / rowsum[c]).
                # One batched reciprocal per head (DVE reciprocal is serial
                # per lane: 32x [1,512] cost 107us, 8x [4,512] cost 27us).
                # The LAST head runs per-chunk so chunk 0 normalizes while
                # chunks 1-3 still compute, letting the projection start early.
                # Broadcast = DMA the reciprocal rows to DRAM, read each back
                # with a stride-0 partition step (legal on the DRAM side
                # only; the custom gpsimd/dve broadcast ops and DVE divide
                # sim fine but are broken/rejected on HW).
                if l < HL - 1:
                    rec32 = smpool.tile([4, 512], FP32, tag="rec32")
                    nc.vector.reciprocal(out=rec32[:], in_=rsg32[:])
                    rec16 = smpool.tile([4, 512], FP16, tag="rec16")
                    nc.vector.tensor_copy(rec16[:], rec32[:])
                    nc.sync.dma_start(out=rscratch[l * 4 : l * 4 + 4], in_=rec16[:])
                for c in range(NT):
                    srow = rscratch[l * 4 + c]
                    bc_sb = smpool.tile([64, 512], FP16, tag="bcsb")
                    nc.sync.dma_start(
                        out=bc_sb[:],
                        in_=bass.AP(
                            tensor=srow.tensor, offset=srow.offset,
                            ap=[[0, 64], [1, 512]],
                        ),
                    )
                    fullq = slice(c * 512, (c + 1) * 512)
                    if l % 2 == 0:
                        nc.vector.tensor_mul(
                            out=y_sb[0:64, mq, fullq], in0=yevs[c][0:64, :], in1=bc_sb[:],
                        )
                    else:
                        ytmp = smpool.tile([64, 512], FP16, tag="ytmp")
                        nc.vector.tensor_mul(out=ytmp[:], in0=yevs[c][0:64, :], in1=bc_sb[:])
                        nc.sync.dma_start(out=y_sb[64:128, mq, fullq], in_=ytmp[:])

            # ---- stage 3: partial output projection ----
            for tch in range(NT):
                tsl = slice(tch * 512, (tch + 1) * 512)
                for mo in range(C // 128):
                    osl = slice(mo * 128, (mo + 1) * 128)
                    pp = psav.tile([128, 512], FP32, tag="pav")
                    for kc in range(MC):
                        nc.tensor.matmul(
                            pp[:],
                            wp_sb[:, kc, osl],
                            y_sb[:, kc, tsl],
                            start=(kc == 0),
                            stop=(kc == MC - 1),
                        )
                    yo_sb = tr.tile([128, 512], FP32, tag="yo")
                    nc.vector.tensor_copy(yo_sb[:], pp[:])
                    nc.sync.dma_start(out=yp[osl, tsl], in_=yo_sb[:])

    nc.compile()
    return nc


_NC = None
LAST_RESULTS = None


def _get_program():
    global _NC
    if _NC is None:
        _NC = _build_program()
    return _NC


# Bucket b covers distances d in [starts[b], starts[b+1]); verified bit-exact
# against the jax reference's _relative_position_bucket for T=2048.
_BUCKET_STARTS = np.array(
    [0, 1, 2, 3, 4, 5, 6, 7, 8, 9, 10, 11, 12, 13, 14, 15,
     16, 18, 20, 23, 26, 29, 33, 38, 43, 49, 55, 63, 72, 82, 93, 106]
)


def _rel_bias_buckets():
    """bucket(d) for d = q - k in [0, T)."""
    d = np.arange(T)
    return np.searchsorted(_BUCKET_STARTS, d, side="right") - 1


def _make_in_maps(x, W_attn, b_attn, W_proj, rel_emb):
    buckets = _rel_bias_buckets()  # [T]
    bias_by_dist = rel_emb[buckets, :]  # [T, H] fp32
    # vec[h, j] = exp(bias[j - 511]) for j >= 511 else 0   (j - 511 = distance d)
    vec = np.zeros((H, EA_VEC), dtype=np.float32)
    vec[:, 511 : 511 + T] = np.exp(bias_by_dist.T)
    vec = vec.astype(np.float16)
    # expand to the per-head Toeplitz table A[h, p, x] = vec[h, x - p + 127]
    sw = np.lib.stride_tricks.sliding_window_view(vec, EA_W, axis=1)  # [H, 128, EA_W]
    wexp_all = np.ascontiguousarray(sw[:, ::-1, :])

    in_maps = []
    for core in range(NCORES):
        b, hg = core // 2, core % 2
        csl = slice(hg * CL, (hg + 1) * CL)
        in_maps.append(
            {
                "xT": np.ascontiguousarray(x[b].T).astype(np.float16),
                "wq": np.ascontiguousarray(W_attn[csl, :].T).astype(np.float16),
                "wk": np.ascontiguousarray(W_attn[C + hg * CL : C + (hg + 1) * CL, :].T).astype(np.float16),
                "wv": np.ascontiguousarray(W_attn[2 * C + hg * CL : 2 * C + (hg + 1) * CL, :].T).astype(np.float16),
                "wp": np.ascontiguousarray(W_proj[:, csl].T).astype(np.float16),
                "bqk": np.stack(
                    [b_attn[csl], b_attn[C + hg * CL : C + (hg + 1) * CL]]
                ).astype(np.float32),
                "bvr": np.ascontiguousarray(np.broadcast_to(
                    b_attn[2 * C + hg * CL : 2 * C + (hg + 1) * CL].astype(np.float32), (128, CL)
                )),
                "wexp": np.ascontiguousarray(wexp_all[hg * HL : (hg + 1) * HL]),
            }
        )
    return in_maps


def kernel(x, W_attn, b_attn, W_proj, b_proj, rel_emb):
    x = np.asarray(x)
    W_attn = np.asarray(W_attn)
    b_attn = np.asarray(b_attn)
    W_proj = np.asarray(W_proj)
    b_proj = np.asarray(b_proj)
    rel_emb = np.asarray(rel_emb)

    in_maps = _make_in_maps(x, W_attn, b_attn, W_proj, rel_emb)
    nc = _get_program()
    res = bass_utils.run_bass_kernel_spmd(nc, in_maps, core_ids=list(range(NCORES)))
    global LAST_RESULTS
    LAST_RESULTS = res

    y = np.empty((B, T, C), dtype=np.float32)
    for b in range(B):
        ypT = res.results[2 * b]["yp"] + res.results[2 * b + 1]["yp"]
        y[b] = ypT.T + b_proj[None, :].astype(np.float32)
    return y



# revision 11
# speedup vs baseline: 1.4913x; 1.4913x over previous
"""Trainium2 Bass kernel for causal self-attention with T5 relative position bias.

Problem (hardcoded): B=4, T=2048, C=1024, H=16, D=64, NUM_BUCKETS=32, MAX_DISTANCE=128.
Sharding over 8 cores: core c -> (batch b=c//2, head-group hg=c%2 of 8 heads).
Each core computes qkv projection for its heads, causal attention, and a partial
output projection (its heads' rows of W_proj); host sums the two partials per batch.

Key structure (v2):
  - Heads are processed in PAIRS (2m on partitions 0-63, 2m+1 on 64-127). The
    two K=64 S-matmuls of a pair target PE row-groups (0,0) and (64,0), so the
    tensor engine runs them CONCURRENTLY (row tiling).
  - AV is "flipped": P tiles [tk,128tq] are the stationary operand, v+ones
    [tk,65] the moving operand -> out [tq,65] costs 65 cycles instead of 512,
    and the softmax rowsum lands as a per-partition COLUMN, so normalization is
    a parallel DVE reciprocal + tensor_scalar multiply (no DRAM bounce).
  - The T5 bias table is host-divided by exp(b31) (bucket 31 = all d >= 106),
    which makes the table exactly 1.0 for far tiles: those skip the DVE
    mask-multiply entirely, and the per-head exp(b31) factor cancels in the
    softmax ratio.
  - Normalized y tiles [tq,128] (head pair side by side) are PE-transposed back
    to [c',tq] for the output projection.
  - Emission is chunk-pipelined: qkv of chunk c+1 and proj of chunk c-1 are
    emitted as PE fillers between S and AV inside attention of chunk c, so the
    tensor engine streams while the scalar engine runs the exps.
"""

import sys

sys.path.insert(0, "/opt/trn_rl_repo")

import math
from collections import deque

import numpy as np

import concourse.bacc as bacc
import concourse.bass as bass
import concourse.mybir as mybir
import concourse.tile as tile
from concourse import bass_utils


def _ensure_axon_hooks():
    """bass_utils imports antenv.axon_hooks when BASS_TRACE is set under axon;
    this image's antenv lacks that submodule. Provide an inert one so a stray
    trace env var degrades to a warning instead of crashing the run."""
    try:
        import antenv.axon_hooks  # noqa: F401
    except Exception:
        try:
            import types

            import antenv

            hooks = types.ModuleType("antenv.axon_hooks")
            hooks._hook = None
            hooks.set_axon_ntff_profile_hook = lambda h: setattr(hooks, "_hook", h)
            hooks.get_axon_ntff_profile_hook = lambda: hooks._hook
            sys.modules["antenv.axon_hooks"] = hooks
            antenv.axon_hooks = hooks
        except Exception:
            pass


_ensure_axon_hooks()

B, T, C = 4, 2048, 1024
H, D = 16, 64
NUM_BUCKETS, MAX_DISTANCE = 32, 128
HL = 8  # local heads per core
CL = HL * D  # 512 local channels
NCORES = 8
NPAIR = HL // 2  # 4 head pairs per core

FP16 = mybir.dt.float16
FP32 = mybir.dt.float32

NT = T // 512  # 4 tq chunks of 512
NK = T // 128  # 16 tk tiles of 128
KC = C // 128  # 8 contraction chunks for qkv
MC = CL // 128  # 4 m-chunks of local channels

# ea table geometry: slice start s = (tq0 - tk0) + 384 in [0, 1920], width 512
EA_W = 2432


def _build_program():
    nc = bacc.Bacc(None, target_bir_lowering=False)

    xT = nc.dram_tensor("xT", [C, T], FP16, kind="ExternalInput")
    wq = nc.dram_tensor("wq", [C, CL], FP16, kind="ExternalInput")
    wk = nc.dram_tensor("wk", [C, CL], FP16, kind="ExternalInput")
    wv = nc.dram_tensor("wv", [C, CL], FP16, kind="ExternalInput")
    wp = nc.dram_tensor("wp", [CL, C], FP16, kind="ExternalInput")
    bqk = nc.dram_tensor("bqk", [2, CL], FP32, kind="ExternalInput")
    bvr = nc.dram_tensor("bvr", [128, CL], FP32, kind="ExternalInput")
    # per-PAIR tables: [pair, head-in-pair, 128, EA_W], host-divided by exp(b31)
    wexp = nc.dram_tensor("wexp", [NPAIR, 2, 128, EA_W], FP16, kind="ExternalInput")
    ident = nc.dram_tensor("ident", [128, 128], FP16, kind="ExternalInput")
    yp = nc.dram_tensor("yp", [C, T], FP16, kind="ExternalOutput")

    import os

    DEBUG = os.environ.get("KDEBUG", "0") == "1"
    if DEBUG:
        d_q = nc.dram_tensor("d_q", [MC, 128, T], FP16, kind="ExternalOutput")
        d_k = nc.dram_tensor("d_k", [MC, 128, T], FP16, kind="ExternalOutput")
        d_v = nc.dram_tensor("d_v", [NK, 128, HL * 65], FP16, kind="ExternalOutput")
        d_y = nc.dram_tensor("d_y", [MC, 128, T], FP16, kind="ExternalOutput")
        d_ps = nc.dram_tensor("d_ps", [NT, 128, 1024], FP32, kind="ExternalOutput")
        d_pm = nc.dram_tensor("d_pm", [NT, 128, 1024], FP16, kind="ExternalOutput")
        d_pav = nc.dram_tensor("d_pav", [NT, 2, 128, 512], FP32, kind="ExternalOutput")

    with tile.TileContext(nc) as tc:
        with (
            tc.tile_pool(name="w", bufs=1) as wpool,
            tc.tile_pool(name="big", bufs=1) as bigpool,
            tc.tile_pool(name="ea", bufs=1) as eapool,
            tc.tile_pool(name="p2", bufs=3) as p2pool,
            tc.tile_pool(name="pm", bufs=3) as pmpool,
            tc.tile_pool(name="sm", bufs=2) as smpool,
            tc.tile_pool(name="yo", bufs=2) as yopool,
            tc.tile_pool(name="ps", bufs=2, space="PSUM") as ps,
            tc.tile_pool(name="pav", bufs=2, space="PSUM") as pavp,
            tc.tile_pool(name="misc", bufs=2, space="PSUM") as miscp,
        ):
            # ---- weights / constants ----
            wq_sb = wpool.tile([128, KC, CL], FP16)
            wk_sb = wpool.tile([128, KC, CL], FP16)
            wv_sb = wpool.tile([128, KC, CL], FP16)
            wp_sb = wpool.tile([128, MC, C], FP16)
            bq_sb = wpool.tile([128, MC], FP32)
            bk_sb = wpool.tile([128, MC], FP32)
            bv_sb = wpool.tile([128, CL], FP32)
            id_sb = wpool.tile([128, 128], FP16)
            xt_sb = bigpool.tile([128, KC, T], FP16)
            ea_sb = [
                eapool.tile([128, 2, EA_W], FP16, name=f"ea{p}") for p in range(NPAIR)
            ]

            xr = xT.rearrange("(kc p) (tc t) -> p kc tc t", p=128, t=512)
            bqk_r = bqk.rearrange("b (m p) -> b p m", p=128)
            wq_r = wq.rearrange("(kc p) m -> p kc m", p=128)

            # kc-granular first loads: the first matmul only waits for its
            # own 128KB slices instead of two 1MB transfers
            nc.sync.dma_start(out=bq_sb, in_=bqk_r[0])
            for kc in range(KC):
                nc.sync.dma_start(out=xt_sb[:, kc, 0:512], in_=xr[:, kc, 0])
                nc.sync.dma_start(out=wq_sb[:, kc], in_=wq_r[:, kc])
            nc.sync.dma_start(out=wk_sb, in_=wk.rearrange("(kc p) m -> p kc m", p=128))
            nc.sync.dma_start(out=bk_sb, in_=bqk_r[1])
            nc.sync.dma_start(out=wv_sb, in_=wv.rearrange("(kc p) m -> p kc m", p=128))
            nc.sync.dma_start(out=bv_sb, in_=bvr[:])
            for tch in range(1, NT):
                nc.sync.dma_start(
                    out=xt_sb[:, :, tch * 512 : (tch + 1) * 512], in_=xr[:, :, tch]
                )
            nc.sync.dma_start(out=id_sb, in_=ident[:])
            wexp_r = wexp.rearrange("pr h p w -> pr p h w")
            for p in range(NPAIR):
                nc.sync.dma_start(out=ea_sb[p], in_=wexp_r[p])
            nc.sync.dma_start(out=wp_sb, in_=wp.rearrange("(kc p) m -> p kc m", p=128))

            # ---- persistent activations ----
            qT_sb = bigpool.tile([128, MC, T], FP16)  # c' = m*128 + p
            kT_sb = bigpool.tile([128, MC, T], FP16)
            v_sb = bigpool.tile([128, NK, HL * 65], FP16)  # slot l: [v(64), ones]
            y_sb = bigpool.tile([128, MC, T], FP16)  # y_cat_T, c_in = m*128 + p

            for l in range(HL):
                nc.vector.memset(v_sb[:, :, l * 65 + 64 : l * 65 + 65], 1.0)

            # ---- qkv / proj closures (PE fillers during attention) ----
            def qk_closure(tch, m, w_sb, b_sb, out_sb):
                def emit():
                    tsl = slice(tch * 512, (tch + 1) * 512)
                    msl = slice(m * 128, (m + 1) * 128)
                    pq = miscp.tile([128, 512], FP32, tag="misc")
                    for kc in range(KC):
                        nc.tensor.matmul(
                            pq[:],
                            w_sb[:, kc, msl],
                            xt_sb[:, kc, tsl],
                            start=(kc == 0),
                            stop=(kc == KC - 1),
                        )
                    nc.vector.tensor_scalar_add(
                        out=out_sb[:, m, tsl], in0=pq[:], scalar1=b_sb[:, m : m + 1]
                    )

                return emit

            def v_closure(tch, ts):
                def emit():
                    t16 = tch * 4 + ts
                    pv = miscp.tile([128, 512], FP32, tag="misc")
                    for kc in range(KC):
                        nc.tensor.matmul(
                            pv[:],
                            xt_sb[:, kc, t16 * 128 : (t16 + 1) * 128],
                            wv_sb[:, kc, :],
                            start=(kc == 0),
                            stop=(kc == KC - 1),
                        )
                    # scatter into 65-wide slots (even/odd strided copies) + bias
                    for par in range(2):
                        src = bass.AP(
                            tensor=pv.tensor,
                            offset=pv.offset + par * 64,
                            ap=[pv.ap[0], [128, 4], [1, 64]],
                        )
                        srcb = bass.AP(
                            tensor=bv_sb.tensor,
                            offset=bv_sb.offset + par * 64,
                            ap=[bv_sb.ap[0], [128, 4], [1, 64]],
                        )
                        base = v_sb[:, t16]
                        dst = bass.AP(
                            tensor=base.tensor,
                            offset=base.offset + par * 65,
                            ap=[base.ap[0], [130, 4], [1, 64]],
                        )
                        nc.vector.tensor_add(out=dst, in0=src, in1=srcb)

                return emit

            def proj_closure(tch, mo):
                def emit():
                    tsl = slice(tch * 512, (tch + 1) * 512)
                    osl = slice(mo * 128, (mo + 1) * 128)
                    pp = miscp.tile([128, 512], FP32, tag="misc")
                    for kcm in range(MC):
                        nc.tensor.matmul(
                            pp[:],
                            wp_sb[:, kcm, osl],
                            y_sb[:, kcm, tsl],
                            start=(kcm == 0),
                            stop=(kcm == MC - 1),
                        )
                    yo_sb = yopool.tile([128, 512], FP16, tag="yo")
                    nc.vector.tensor_copy(yo_sb[:], pp[:])
                    nc.sync.dma_start(out=yp[osl, tsl], in_=yo_sb[:])

                return emit

            def qkv_closures(tch):
                # q first (attention chunk tch needs qT before any kT tile)
                cl = [qk_closure(tch, m, wq_sb, bq_sb, qT_sb) for m in range(MC)]
                cl += [qk_closure(tch, m, wk_sb, bk_sb, kT_sb) for m in range(MC)]
                cl += [v_closure(tch, ts) for ts in range(4)]
                return cl

            # ---- attention emission, chunk-pipelined ----
            for cl in qkv_closures(0):
                cl()

            for c in range(NT):
                fillers = deque()
                if c + 1 < NT:
                    fillers.extend(qkv_closures(c + 1))
                if c >= 1:
                    fillers.extend([proj_closure(c - 1, mo) for mo in range(C // 128)])
                nj = 4 * c + 4
                total_iters = NPAIR * nj
                nfill = len(fillers)
                it = 0
                popped = 0

                for pair in range(NPAIR):
                    pavA = pavp.tile([128, 512], FP32, tag="pav")
                    pavB = pavp.tile([128, 512], FP32, tag="pav")
                    for j in range(nj):
                        off = max(0, 128 * j - 512 * c)  # multiple of 128
                        far = j <= 4 * c - 2
                        s = 512 * c - 128 * j + 384

                        pS = ps.tile([128, 1024], FP32, tag="pS")
                        nc.tensor.matmul(
                            pS[:, off:512],
                            kT_sb[0:64, pair, j * 128 : (j + 1) * 128],
                            qT_sb[0:64, pair, c * 512 + off : (c + 1) * 512],
                            start=True,
                            stop=True,
                        )
                        nc.tensor.matmul(
                            pS[:, 512 + off : 1024],
                            kT_sb[64:128, pair, j * 128 : (j + 1) * 128],
                            qT_sb[64:128, pair, c * 512 + off : (c + 1) * 512],
                            start=True,
                            stop=True,
                        )

                        # PE fillers go between S and AV so the tensor engine
                        # streams while ACT computes the exp
                        it += 1
                        due = nfill * it // total_iters
                        while popped < due:
                            fillers.popleft()()
                            popped += 1

                        p2 = p2pool.tile([128, 1024], FP16, tag="p2")
                        nc.scalar.activation(
                            out=p2[:, off:1024],
                            in_=pS[:, off:1024],
                            func=mybir.ActivationFunctionType.Exp,
                            scale=1.0 / math.sqrt(D),
                        )
                        if far:
                            pmt = p2
                        else:
                            pmt = pmpool.tile([128, 1024], FP16, tag="pm")
                            ea_pair = bass.AP(
                                tensor=ea_sb[pair].tensor,
                                offset=ea_sb[pair].offset + s + off,
                                ap=[ea_sb[pair].ap[0], [EA_W, 2], [1, 512 - off]],
                            )
                            dst = bass.AP(
                                tensor=pmt.tensor,
                                offset=pmt.offset + off,
                                ap=[pmt.ap[0], [512, 2], [1, 512 - off]],
                            )
                            srcp = bass.AP(
                                tensor=p2.tensor,
                                offset=p2.offset + off,
                                ap=[p2.ap[0], [512, 2], [1, 512 - off]],
                            )
                            nc.vector.tensor_mul(out=dst, in0=srcp, in1=ea_pair)

                        if DEBUG and pair == 0 and j == 4 * c + 1:
                            dps = smpool.tile([128, 1024], FP32, tag="dps")
                            nc.vector.tensor_copy(dps[:], pS[:])
                            nc.sync.dma_start(out=d_ps[c], in_=dps[:])
                            nc.sync.dma_start(out=d_pm[c], in_=pmt[:])

                        for k in range(off // 128, 4):
                            # start=True clears the has_written map of the
                            # WHOLE PSUM bank, and start=False overwrites
                            # where bits are clear — so only the bank's
                            # first matmul of the pair-chunk may set start.
                            # stop closes each subtile's group at its true
                            # last contribution (j == 4c+k; sim-only).
                            nc.tensor.matmul(
                                pavA[:, k * 65 : k * 65 + 65],
                                pmt[:, k * 128 : (k + 1) * 128],
                                v_sb[:, j, 2 * pair * 65 : 2 * pair * 65 + 65],
                                start=(j == 0 and k == 0),
                                stop=(j == 4 * c + k),
                                skip_group_check=True,
                            )
                            nc.tensor.matmul(
                                pavB[:, k * 65 : k * 65 + 65],
                                pmt[:, 512 + k * 128 : 512 + (k + 1) * 128],
                                v_sb[:, j, (2 * pair + 1) * 65 : (2 * pair + 1) * 65 + 65],
                                start=(j == 0 and k == 0),
                                stop=(j == 4 * c + k),
                                skip_group_check=True,
                            )

                    # ---- tail: normalize + transpose + evacuate ----
                    if DEBUG and pair == 0:
                        for hh, pv in ((0, pavA), (1, pavB)):
                            dpa = smpool.tile([128, 512], FP32, tag="dpa")
                            nc.vector.tensor_copy(dpa[:], pv[:])
                            nc.sync.dma_start(out=d_pav[c, hh], in_=dpa[:])
                    rec = smpool.tile([128, 8], FP32, tag="rec")
                    recA = bass.AP(
                        tensor=pavA.tensor,
                        offset=pavA.offset + 64,
                        ap=[pavA.ap[0], [65, 4]],
                    )
                    recB = bass.AP(
                        tensor=pavB.tensor,
                        offset=pavB.offset + 64,
                        ap=[pavB.ap[0], [65, 4]],
                    )
                    nc.vector.reciprocal(out=rec[:, 0:4], in_=recA)
                    nc.vector.reciprocal(out=rec[:, 4:8], in_=recB)
                    yn = smpool.tile([128, 4, 128], FP16, tag="yn")
                    for k in range(4):
                        nc.vector.tensor_scalar_mul(
                            out=yn[:, k, 0:64],
                            in0=pavA[:, k * 65 : k * 65 + 64],
                            scalar1=rec[:, k : k + 1],
                        )
                        nc.vector.tensor_scalar_mul(
                            out=yn[:, k, 64:128],
                            in0=pavB[:, k * 65 : k * 65 + 64],
                            scalar1=rec[:, 4 + k : 5 + k],
                        )
                    pT = miscp.tile([128, 512], FP16, tag="misc")
                    for k in range(4):
                        nc.tensor.transpose(
                            out=pT[:, k * 128 : (k + 1) * 128],
                            in_=yn[:, k],
                            identity=id_sb[:],
                        )
                    nc.vector.tensor_copy(
                        y_sb[:, pair, c * 512 : (c + 1) * 512], pT[:]
                    )

                while fillers:
                    fillers.popleft()()

            for mo in range(C // 128):
                proj_closure(NT - 1, mo)()

            if DEBUG:
                for m in range(MC):
                    nc.sync.dma_start(out=d_q[m], in_=qT_sb[:, m, :])
                    nc.sync.dma_start(out=d_k[m], in_=kT_sb[:, m, :])
                    nc.sync.dma_start(out=d_y[m], in_=y_sb[:, m, :])
                for t16 in range(NK):
                    nc.sync.dma_start(out=d_v[t16], in_=v_sb[:, t16, :])

    nc.compile()
    return nc


_NC = None
LAST_RESULTS = None


def _get_program():
    global _NC
    if _NC is None:
        _NC = _build_program()
    return _NC


# Bucket b covers distances d in [starts[b], starts[b+1]); verified bit-exact
# against the jax reference's _relative_position_bucket for T=2048.
_BUCKET_STARTS = np.array(
    [0, 1, 2, 3, 4, 5, 6, 7, 8, 9, 10, 11, 12, 13, 14, 15,
     16, 18, 20, 23, 26, 29, 33, 38, 43, 49, 55, 63, 72, 82, 93, 106]
)


def _rel_bias_buckets():
    """bucket(d) for d = q - k in [0, T)."""
    d = np.arange(T)
    return np.searchsorted(_BUCKET_STARTS, d, side="right") - 1


def _make_in_maps(x, W_attn, b_attn, W_proj, rel_emb):
    buckets = _rel_bias_buckets()  # [T]
    bias_by_dist = rel_emb[buckets, :]  # [T, H] fp32
    # Divide by exp(b31) per head: far tiles (all d >= 106, bucket 31) then
    # multiply by exactly 1.0 and can skip the mask-multiply; the factor
    # cancels in the softmax ratio.
    b31 = rel_emb[NUM_BUCKETS - 1, :]  # [H]
    # vec[h, j] = exp(bias[j - 511] - b31[h]) for j >= 511 else 0
    vec = np.zeros((H, EA_W + 127), dtype=np.float32)
    vec[:, 511 : 511 + T] = np.exp(bias_by_dist.T - b31[:, None])
    vec = vec.astype(np.float16)
    # expand to the per-head Toeplitz table A[h, p, x] = vec[h, x - p + 127]
    sw = np.lib.stride_tricks.sliding_window_view(vec, EA_W, axis=1)  # [H,128,EA_W]
    wexp_all = np.ascontiguousarray(sw[:, ::-1, :])  # [H, 128, EA_W]

    ident = np.eye(128, dtype=np.float16)

    in_maps = []
    for core in range(NCORES):
        b, hg = core // 2, core % 2
        csl = slice(hg * CL, (hg + 1) * CL)
        in_maps.append(
            {
                "xT": np.ascontiguousarray(x[b].T).astype(np.float16),
                "wq": np.ascontiguousarray(W_attn[csl, :].T).astype(np.float16),
                "wk": np.ascontiguousarray(
                    W_attn[C + hg * CL : C + (hg + 1) * CL, :].T
                ).astype(np.float16),
                "wv": np.ascontiguousarray(
                    W_attn[2 * C + hg * CL : 2 * C + (hg + 1) * CL, :].T
                ).astype(np.float16),
                "wp": np.ascontiguousarray(W_proj[:, csl].T).astype(np.float16),
                "bqk": np.stack(
                    [b_attn[csl], b_attn[C + hg * CL : C + (hg + 1) * CL]]
                ).astype(np.float32),
                "bvr": np.ascontiguousarray(
                    np.broadcast_to(
                        b_attn[2 * C + hg * CL : 2 * C + (hg + 1) * CL].astype(
                            np.float32
                        ),
                        (128, CL),
                    )
                ),
                "wexp": np.ascontiguousarray(
                    wexp_all[hg * HL : (hg + 1) * HL].reshape(NPAIR, 2, 128, EA_W)
                ),
                "ident": ident,
            }
        )
    return in_maps


def kernel(x, W_attn, b_attn, W_proj, b_proj, rel_emb):
    x = np.asarray(x)
    W_attn = np.asarray(W_attn)
    b_attn = np.asarray(b_attn)
    W_proj = np.asarray(W_proj)
    b_proj = np.asarray(b_proj)
    rel_emb = np.asarray(rel_emb)

    in_maps = _make_in_maps(x, W_attn, b_attn, W_proj, rel_emb)
    nc = _get_program()
    res = bass_utils.run_bass_kernel_spmd(nc, in_maps, core_ids=list(range(NCORES)))
    global LAST_RESULTS
    LAST_RESULTS = res

    y = np.empty((B, T, C), dtype=np.float32)
    for b in range(B):
        ypT = res.results[2 * b]["yp"].astype(np.float32) + res.results[2 * b + 1][
            "yp"
        ].astype(np.float32)
        y[b] = ypT.T + b_proj[None, :].astype(np.float32)
    return y
